# revision 1
# baseline (speedup 1.0000x reference)
"""Trainium2 Bass kernel for nn_InputLayer_57337813401914.

out[b, n, v] = sin(x[b, n] * freqs[v]), x: [64, 4096] f32,
freqs[v] = 10 ** (min(v, 127) / 127 * 4), v in [0, 256).

Sharding: batch dim (64) split across 8 NeuronCores. Per core the 32768
x-values sit [128 partitions, 256 free] in natural order; the kernel is
freq-major: each DVE tensor_scalar multiplies the whole x tile by one
frequency immediate (exact f32 product), groups of 8 frequencies share one
fused rint pass (DVE mult+add against the 1.5*2^23 magic), n-extraction on
GPSIMD, a 3-term Cody-Waite cascade (single custom DVE op) reduces the phase
into [-pi-eps, pi+eps], and the ACT Sin table (4-ULP on that domain) produces
the result. Device output is freq-major [256, 32768]; rows 127..255 all equal
sin(1e4 * x), so rows 128..255 are written by stride-0-source replicate DMAs
from the row-127 tile (computed in the first group so the 16 MiB of replicate
traffic overlaps the remaining compute). Host transposes back to [.., 256].
"""
import numpy as np
from contextlib import ExitStack

import concourse.bacc as bacc
import concourse.tile as tile
from concourse import mybir
from concourse.alu_op_type import AluOpType as A
from concourse.bass_utils import run_bass_kernel_spmd

P = 128          # SBUF partitions
NXF = 256        # x values per partition (free dim)
NJ = P * NXF     # 32768 x values per core
VLEAD = 128      # computed frequency rows (row 127 == 1e4 == rows 128..255)
V = 256          # total output rows (freq-major)
FG = 8           # frequencies per instruction group
NGROUPS = VLEAD // FG
NREP = 4         # replicate DMA count for rows 128..255
NCORES = 8
B, N = 64, 4096
B_PER_CORE = B // NCORES

MAGIC = float(np.float32(1.5 * 2**23))
INV_2PI = float(np.float32(1.0 / (2.0 * np.pi)))


def _round_bits(v: float, bits: int) -> np.float32:
    _, e = np.frexp(np.float64(v))
    scale = 2.0 ** (bits - e)
    return np.float32(np.round(np.float64(v) * scale) / scale)


# 3-term Cody-Waite split of 2*pi: n*C1 and n*C2 exact for integer |n| < 2^14.
C1 = _round_bits(2.0 * np.pi, 10)
C2 = _round_bits(2.0 * np.pi - np.float64(C1), 10)
C3 = np.float32(2.0 * np.pi - np.float64(C1) - np.float64(C2))

_BUILT = None


def _freqs_lead() -> np.ndarray:
    """First 128 freqs, computed exactly as the reference does (jnp f32 ops on
    the default jax backend so the bits match the grader's reference)."""
    try:
        import jax.numpy as jnp

        f = 10.0 ** (jnp.arange(VLEAD, dtype=jnp.float32) / (VLEAD - 1) * 4.0)
        return np.asarray(f, dtype=np.float32)
    except Exception:
        y = np.arange(VLEAD, dtype=np.float32) / np.float32(VLEAD - 1)
        y = y * np.float32(4.0)
        return np.power(np.float32(10.0), y, dtype=np.float32)


def _build(freqs: np.ndarray, reps: int = 1):
    nc = bacc.Bacc(
        "TRN2", target_bir_lowering=False, debug=False, num_devices=NCORES
    )
    f32 = mybir.dt.float32
    xT_in = nc.dram_tensor("xT", [P, NXF], f32, kind="ExternalInput").ap()
    out_t = nc.dram_tensor("out", [V, NJ], f32, kind="ExternalOutput").ap()
    # out[v, p*256 + i] viewed with partition dim first:
    out_v = out_t.rearrange("v (p i) -> p v i", p=P)

    # First group is freq 127 ALONE: its sin tile feeds row 127's store plus
    # the replicate DMAs for rows 128..255, so DMA traffic starts ~2.5us in
    # and the 16 MiB of replication overlaps all remaining compute.
    groups = [[VLEAD - 1]]
    for v0 in range(0, VLEAD - 1, FG):
        groups.append(list(range(v0, min(v0 + FG, VLEAD - 1))))

    with tile.TileContext(nc) as tc:
        with ExitStack() as ctx:
            const = ctx.enter_context(tc.tile_pool(name="const", bufs=1))
            work = ctx.enter_context(tc.tile_pool(name="work", bufs=3))
            outp = ctx.enter_context(tc.tile_pool(name="outp", bufs=4))

            xT = const.tile([P, NXF], f32, tag="xT")
            nc.sync.dma_start(xT[:], xT_in[:])
            s127_tile = None

            # reps > 1 repeats the whole pipeline over the same output for
            # wall-clock HW timing via (T_reps - T_1) / (reps - 1)
            for gi, vlist in enumerate(groups * reps):
                g = len(vlist)
                t_big = work.tile([P, g * NXF], f32, tag="t")
                for k in range(g):
                    nc.vector.tensor_scalar(
                        t_big[:, k * NXF : (k + 1) * NXF],
                        xT[:],
                        float(freqs[vlist[k]]),
                        None,
                        A.mult,
                    )
                # w = t*inv2pi + magic  =>  w - magic = rint(t/2pi)
                w_big = work.tile([P, g * NXF], f32, tag="w")
                nc.vector.tensor_scalar(
                    w_big[:], t_big[:], INV_2PI, MAGIC, A.mult, A.add
                )
                n_big = work.tile([P, g * NXF], f32, tag="n")
                nc.gpsimd.tensor_scalar(n_big[:], w_big[:], MAGIC, None, A.subtract)
                r_big = work.tile([P, g * NXF], f32, tag="r")
                nc.vector.cody_waite_cascade(
                    r_big[:], t_big[:], n_big[:], float(C1), float(C2), float(C3)
                )
                if gi == 0:
                    # sin(1e4 * x): lives in the const pool for the replicates
                    s_big = const.tile([P, NXF], f32, tag="s127")
                else:
                    s_big = outp.tile([P, g * NXF], f32, tag="s")
                nc.scalar.activation(
                    s_big[:], r_big[:], mybir.ActivationFunctionType.Sin
                )
                nc.sync.dma_start(
                    out_v[:, vlist[0] : vlist[0] + g, :],
                    s_big[:].rearrange("p (k i) -> p k i", i=NXF),
                )
                if gi == 0:
                    s127_tile = s_big
                # rows 128..255 replicate row 127: spread NREP DMAs across the
                # run (one every few groups) on the ACT HWDGE ring so the SP
                # store ring isn't head-of-line blocked and the DMA engines
                # alternate between stores and replication.
                rep_every = max(1, len(groups) // NREP)
                if gi % rep_every == 0 and gi // rep_every < NREP:
                    r = gi // rep_every
                    rep_rows = (V - VLEAD) // NREP
                    r0 = VLEAD + r * rep_rows
                    nc.scalar.dma_start(
                        out_v[:, r0 : r0 + rep_rows, :],
                        s127_tile[:].unsqueeze(1).to_broadcast([P, rep_rows, NXF]),
                    )

    nc.compile()
    return nc


def kernel(x, vector_size):
    global _BUILT
    x = np.asarray(x, dtype=np.float32)
    assert x.shape == (B, N), x.shape
    assert int(vector_size) == V, vector_size

    freqs = _freqs_lead()
    if _BUILT is None:
        _BUILT = _build(freqs)
    nc = _BUILT

    in_maps = []
    for c in range(NCORES):
        xs = x[c * B_PER_CORE : (c + 1) * B_PER_CORE].reshape(P, NXF)
        in_maps.append({"xT": np.ascontiguousarray(xs)})

    res = run_bass_kernel_spmd(nc, in_maps, list(range(NCORES)))

    out = np.empty((B, N, V), dtype=np.float32)
    for c in range(NCORES):
        oc = res.results[c]["out"]  # [256, 32768] freq-major
        out[c * B_PER_CORE : (c + 1) * B_PER_CORE] = np.ascontiguousarray(
            oc.T
        ).reshape(B_PER_CORE, N, V)
    return out



# revision 5
# speedup vs baseline: 3.9524x; 3.9524x over previous
"""Trainium2 Bass kernel for nn_InputLayer_57337813401914.

out[b, n, v] = sin(x[b, n] * freqs[v]), x: [64, 4096] f32,
freqs[v] = 10 ** (min(v, 127) / 127 * 4), v in [0, 256).

Sharding: batch dim (64) split across 8 NeuronCores; per core 32768 x
values, device output freq-major [256, 32768] (host transposes back).

Per-core pipeline (partition = frequency):
  PE    u[v, j]  = (freqs[v]/2pi) * x[j]      64 outer products -> PSUM
  ACT   w2       = Identity(-u + MAGIC)       = MAGIC - rint(u) (magic trick)
  DVE   r        = (w2 - MAGIC) + u           = u - rint(u) in [-0.5, 0.5]
  ACT   s        = Sin(2pi * r)               table exact on [-pi, pi]
  DMA   out[v, chunk] <- s                    sync ring
Rows 127..255 all equal sin(1e4 * x): a tiny [128, 256]-layout side
pipeline computes s127 once and four broadcast DMAs on the scalar ring
replicate it into rows 128..255, overlapped with the main chunk loop.
Single f32 multiply range reduction: rel err vs reference ~2.7e-4
(tolerance 2e-2); the magic-number rint and the exact Sterbenz
subtractions make r = frac-reduction of fl(x*f/2pi) exact.
"""
import numpy as np
from contextlib import ExitStack

import concourse.bacc as bacc
import concourse.tile as tile
from concourse import mybir
from concourse.alu_op_type import AluOpType as A
from concourse.bass_utils import run_bass_kernel_spmd

P = 128            # SBUF/PSUM partitions; also the number of distinct freqs
NX = 32768         # x values per core
VLEAD = 128        # computed frequency rows (rows 128..255 replicate row 127)
V = 256            # total output rows
J = 2048           # x chunk per pipeline stage (free dim)
NCHUNK = NX // J   # 16
MMN = 512          # moving free dim per fp32 matmul (PE limit / PSUM bank)
NMM = J // MMN     # 4 matmuls per chunk
XROWS = 2          # x halves live on partitions 0 and 32 (matmul base rule)
XCOL = NX // XROWS
NREP = 4           # replicate DMA count for rows 128..255
NCORES = 8
B, N = 64, 4096
B_PER_CORE = B // NCORES

MAGIC = float(np.float32(1.5 * 2**23))
TWO_PI = float(np.float32(2.0 * np.pi))

_BUILT = None


def _freqs_lead() -> np.ndarray:
    """First 128 freqs, bit-matching the reference (jnp f32 ops)."""
    try:
        import jax.numpy as jnp

        f = 10.0 ** (jnp.arange(VLEAD, dtype=jnp.float32) / (VLEAD - 1) * 4.0)
        return np.asarray(f, dtype=np.float32)
    except Exception:
        y = np.arange(VLEAD, dtype=np.float32) / np.float32(VLEAD - 1)
        y = y * np.float32(4.0)
        return np.power(np.float32(10.0), y, dtype=np.float32)


def _freqs_over_2pi() -> np.ndarray:
    return (
        _freqs_lead().astype(np.float64) / (2.0 * np.pi)
    ).astype(np.float32)


def _build(fp: np.ndarray):
    nc = bacc.Bacc(
        "TRN2", target_bir_lowering=False, debug=False, num_devices=NCORES
    )
    f32 = mybir.dt.float32
    x2_in = nc.dram_tensor("x2", [XROWS, XCOL], f32, kind="ExternalInput").ap()
    x128_in = nc.dram_tensor("x128", [P, NX // P], f32, kind="ExternalInput").ap()
    frep_in = nc.dram_tensor("frep", [XROWS, P], f32, kind="ExternalInput").ap()
    out_t = nc.dram_tensor("out", [V, NX], f32, kind="ExternalOutput").ap()
    # out[v, p*256 + i] viewed partition-first: replicate-DMA destination
    out_v = out_t.rearrange("v (p i) -> p v i", p=P)

    fp127 = float(fp[VLEAD - 1])

    with tile.TileContext(nc) as tc:
        with ExitStack() as ctx:
            const = ctx.enter_context(tc.tile_pool(name="const", bufs=1))
            psum = ctx.enter_context(
                tc.tile_pool(name="psum", bufs=2, space="PSUM")
            )
            work = ctx.enter_context(tc.tile_pool(name="work", bufs=3))
            outp = ctx.enter_context(tc.tile_pool(name="outp", bufs=4))

            x2 = const.tile([33, XCOL], f32, tag="x2")
            x128 = const.tile([P, NX // P], f32, tag="x128")
            frep = const.tile([33, P], f32, tag="frep")
            nc.sync.dma_start(x2[0:1, :], x2_in[0:1, :])
            nc.sync.dma_start(x2[32:33, :], x2_in[1:2, :])
            nc.sync.dma_start(x128[:], x128_in[:])
            nc.sync.dma_start(frep[0:1, :], frep_in[0:1, :])
            nc.sync.dma_start(frep[32:33, :], frep_in[1:2, :])
            # per-partition MAGIC column for the activation bias operand
            mgc = const.tile([P, 1], f32, tag="magic")
            nc.vector.memset(mgc[:], MAGIC)
            zro = const.tile([P, 1], f32, tag="zero")
            nc.vector.memset(zro[:], 0.0)

            # side pipeline: s127[p, i] = sin(1e4 * x[p*256+i]) feeds the
            # broadcast replicates for rows 128..255
            u7 = work.tile([P, NX // P], f32, tag="u7")
            nc.vector.tensor_scalar(u7[:], x128[:], fp127, None, A.mult)
            w7 = work.tile([P, NX // P], f32, tag="w7")
            nc.scalar.activation(
                w7[:], u7[:], mybir.ActivationFunctionType.Identity,
                bias=mgc[:], scale=-1.0,
            )
            r7 = work.tile([P, NX // P], f32, tag="r7")
            nc.vector.scalar_tensor_tensor(
                r7[:], w7[:], MAGIC, u7[:], A.subtract, A.add
            )
            s127 = const.tile([P, NX // P], f32, tag="s127")
            nc.scalar.activation(
                s127[:], r7[:], mybir.ActivationFunctionType.Sin,
                bias=zro[:], scale=TWO_PI,
            )

            rep_rows = (V - VLEAD) // NREP
            rep_every = NCHUNK // NREP

            # software-pipelined main loop: ACT queue order is
            # w2(0), w2(1), sin(0), w2(2), sin(1), ... so the per-chunk
            # ACT->DVE->ACT chain overlaps across chunks.
            prev = None  # (r_tile, u_tile, c) pending sin+store
            for c in range(NCHUNK + 1):
                if c < NCHUNK:
                    u_t = psum.tile([P, J], f32, tag="u")
                    for m in range(NMM):
                        g = c * NMM + m
                        row = 32 * (g // (XCOL // MMN))
                        col = (g % (XCOL // MMN)) * MMN
                        nc.tensor.matmul(
                            u_t[:, m * MMN : (m + 1) * MMN],
                            frep[row : row + 1, :],
                            x2[row : row + 1, col : col + MMN],
                            start=True,
                            stop=True,
                        )
                    w2_t = work.tile([P, J], f32, tag="w2")
                    nc.scalar.activation(
                        w2_t[:], u_t[:], mybir.ActivationFunctionType.Identity,
                        bias=mgc[:], scale=-1.0,
                    )
                    r_t = work.tile([P, J], f32, tag="r")
                    nc.vector.scalar_tensor_tensor(
                        r_t[:], w2_t[:], MAGIC, u_t[:], A.subtract, A.add
                    )
                    prev_next = (r_t, c)
                else:
                    prev_next = None

                if prev is not None:
                    r_p, cp = prev
                    s_t = outp.tile([P, J], f32, tag="s")
                    nc.scalar.activation(
                        s_t[:], r_p[:], mybir.ActivationFunctionType.Sin,
                        bias=zro[:], scale=TWO_PI,
                    )
                    nc.sync.dma_start(
                        out_t[0:VLEAD, cp * J : (cp + 1) * J], s_t[:]
                    )
                    # spread the 4 big replicate DMAs across the run on the
                    # scalar HWDGE ring so both rings share HBM evenly
                    if cp % rep_every == 0 and cp // rep_every < NREP:
                        rr = cp // rep_every
                        r0 = VLEAD + rr * rep_rows
                        nc.scalar.dma_start(
                            out_v[:, r0 : r0 + rep_rows, :],
                            s127[:]
                            .unsqueeze(1)
                            .to_broadcast([P, rep_rows, NX // P]),
                        )
                prev = prev_next

    nc.compile()
    return nc


def kernel(x, vector_size):
    global _BUILT
    x = np.asarray(x, dtype=np.float32)
    assert x.shape == (B, N), x.shape
    assert int(vector_size) == V, vector_size

    fp = _freqs_over_2pi()
    if _BUILT is None:
        _BUILT = _build(fp)
    nc = _BUILT

    frep = np.ascontiguousarray(np.tile(fp[None, :], (XROWS, 1)))
    in_maps = []
    for c in range(NCORES):
        xs = x[c * B_PER_CORE : (c + 1) * B_PER_CORE].reshape(-1)
        in_maps.append(
            {
                "x2": np.ascontiguousarray(xs.reshape(XROWS, XCOL)),
                "x128": np.ascontiguousarray(xs.reshape(P, NX // P)),
                "frep": frep,
            }
        )

    res = run_bass_kernel_spmd(nc, in_maps, list(range(NCORES)))

    out = np.empty((B, N, V), dtype=np.float32)
    for c in range(NCORES):
        oc = res.results[c]["out"]  # [256, 32768] freq-major
        out[c * B_PER_CORE : (c + 1) * B_PER_CORE] = np.ascontiguousarray(
            oc.T
        ).reshape(B_PER_CORE, N, V)
    return out


# revision 6
# speedup vs baseline: 4.3814x; 1.1085x over previous
"""Trainium2 Bass kernel for nn_InputLayer_57337813401914.

out[b, n, v] = sin(x[b, n] * freqs[v]), x: [64, 4096] f32,
freqs[v] = 10 ** (min(v, 127) / 127 * 4), v in [0, 256).

Sharding: batch dim (64) split across 8 NeuronCores; per core 32768 x
values, device output freq-major [256, 32768] (host transposes back).

Per-core pipeline (partition = frequency):
  PE    u[v, j]  = (freqs[v]/2pi) * x[j]      64 outer products -> PSUM
  ACT   w2       = Identity(-u + MAGIC)       = MAGIC - rint(u) (magic trick)
  DVE   r        = (w2 - MAGIC) + u           = u - rint(u) in [-0.5, 0.5]
  ACT   s        = Sin(2pi * r)               table exact on [-pi, pi]
  DMA   out[v, chunk] <- s                    sync ring
Rows 127..255 all equal sin(1e4 * x): a tiny [128, 256]-layout side
pipeline computes s127 once and four broadcast DMAs on the scalar ring
replicate it into rows 128..255, overlapped with the main chunk loop.
Single f32 multiply range reduction: rel err vs reference ~2.7e-4
(tolerance 2e-2); the magic-number rint and the exact Sterbenz
subtractions make r = frac-reduction of fl(x*f/2pi) exact.
"""
import numpy as np
from contextlib import ExitStack

import concourse.bacc as bacc
import concourse.tile as tile
from concourse import mybir
from concourse.alu_op_type import AluOpType as A
from concourse.bass_utils import run_bass_kernel_spmd

P = 128            # SBUF/PSUM partitions; also the number of distinct freqs
NX = 32768         # x values per core
VLEAD = 128        # computed frequency rows (rows 128..255 replicate row 127)
V = 256            # total output rows
J = 2048           # x chunk per pipeline stage (free dim)
NCHUNK = NX // J   # 16
MMN = 512          # moving free dim per fp32 matmul (PE limit / PSUM bank)
NMM = J // MMN     # 4 matmuls per chunk
XROWS = 2          # x halves live on partitions 0 and 32 (matmul base rule)
XCOL = NX // XROWS
KSP = 6            # bf16 split products per outer product (see _split6)
NREP = 4           # replicate DMA count for rows 128..255
NCORES = 8
B, N = 64, 4096
B_PER_CORE = B // NCORES

MAGIC = float(np.float32(1.5 * 2**23))
TWO_PI = float(np.float32(2.0 * np.pi))

_BUILT = None


def _freqs_lead() -> np.ndarray:
    """First 128 freqs, bit-matching the reference (jnp f32 ops)."""
    try:
        import jax.numpy as jnp

        f = 10.0 ** (jnp.arange(VLEAD, dtype=jnp.float32) / (VLEAD - 1) * 4.0)
        return np.asarray(f, dtype=np.float32)
    except Exception:
        y = np.arange(VLEAD, dtype=np.float32) / np.float32(VLEAD - 1)
        y = y * np.float32(4.0)
        return np.power(np.float32(10.0), y, dtype=np.float32)


def _freqs_over_2pi() -> np.ndarray:
    return (
        _freqs_lead().astype(np.float64) / (2.0 * np.pi)
    ).astype(np.float32)


def _build(fp: np.ndarray):
    nc = bacc.Bacc(
        "TRN2", target_bir_lowering=False, debug=False, num_devices=NCORES
    )
    f32 = mybir.dt.float32
    bf16 = mybir.dt.bfloat16
    x6_in = nc.dram_tensor(
        "x6", [XROWS * KSP, XCOL], bf16, kind="ExternalInput"
    ).ap()
    x128_in = nc.dram_tensor("x128", [P, NX // P], f32, kind="ExternalInput").ap()
    f6_in = nc.dram_tensor(
        "f6", [XROWS * KSP, P], bf16, kind="ExternalInput"
    ).ap()
    out_t = nc.dram_tensor("out", [V, NX], f32, kind="ExternalOutput").ap()
    # out[v, p*256 + i] viewed partition-first: replicate-DMA destination
    out_v = out_t.rearrange("v (p i) -> p v i", p=P)

    fp127 = float(fp[VLEAD - 1])

    with tile.TileContext(nc) as tc:
        with ExitStack() as ctx:
            const = ctx.enter_context(tc.tile_pool(name="const", bufs=1))
            psum = ctx.enter_context(
                tc.tile_pool(name="psum", bufs=2, space="PSUM")
            )
            work = ctx.enter_context(tc.tile_pool(name="work", bufs=3))
            outp = ctx.enter_context(tc.tile_pool(name="outp", bufs=4))

            x6 = const.tile([32 + KSP, XCOL], bf16, tag="x6")
            x128 = const.tile([P, NX // P], f32, tag="x128")
            f6 = const.tile([32 + KSP, P], bf16, tag="f6")
            nc.sync.dma_start(x6[0:KSP, :], x6_in[0:KSP, :])
            nc.sync.dma_start(x6[32 : 32 + KSP, :], x6_in[KSP : 2 * KSP, :])
            nc.sync.dma_start(x128[:], x128_in[:])
            nc.sync.dma_start(f6[0:KSP, :], f6_in[0:KSP, :])
            nc.sync.dma_start(f6[32 : 32 + KSP, :], f6_in[KSP : 2 * KSP, :])
            # per-partition MAGIC column for the activation bias operand
            mgc = const.tile([P, 1], f32, tag="magic")
            nc.vector.memset(mgc[:], MAGIC)
            zro = const.tile([P, 1], f32, tag="zero")
            nc.vector.memset(zro[:], 0.0)

            # side pipeline: s127[p, i] = sin(1e4 * x[p*256+i]) feeds the
            # broadcast replicates for rows 128..255
            u7 = work.tile([P, NX // P], f32, tag="u7")
            nc.vector.tensor_scalar(u7[:], x128[:], fp127, None, A.mult)
            w7 = work.tile([P, NX // P], f32, tag="w7")
            nc.scalar.activation(
                w7[:], u7[:], mybir.ActivationFunctionType.Identity,
                bias=mgc[:], scale=-1.0,
            )
            r7 = work.tile([P, NX // P], f32, tag="r7")
            nc.vector.scalar_tensor_tensor(
                r7[:], w7[:], MAGIC, u7[:], A.subtract, A.add
            )
            s127 = const.tile([P, NX // P], f32, tag="s127")
            nc.scalar.activation(
                s127[:], r7[:], mybir.ActivationFunctionType.Sin,
                bias=zro[:], scale=TWO_PI,
            )

            rep_rows = (V - VLEAD) // NREP
            rep_every = NCHUNK // NREP

            # software-pipelined main loop: ACT queue order is
            # w2(0), w2(1), sin(0), w2(2), sin(1), ... so the per-chunk
            # ACT->DVE->ACT chain overlaps across chunks.
            prev = None  # (r_tile, u_tile, c) pending sin+store
            for c in range(NCHUNK + 1):
                if c < NCHUNK:
                    u_t = psum.tile([P, J], f32, tag="u")
                    for m in range(NMM):
                        g = c * NMM + m
                        row = 32 * (g // (XCOL // MMN))
                        col = (g % (XCOL // MMN)) * MMN
                        nc.tensor.matmul(
                            u_t[:, m * MMN : (m + 1) * MMN],
                            f6[row : row + KSP, :],
                            x6[row : row + KSP, col : col + MMN],
                            start=True,
                            stop=True,
                        )
                    w2_t = work.tile([P, J], f32, tag="w2")
                    nc.scalar.activation(
                        w2_t[:], u_t[:], mybir.ActivationFunctionType.Identity,
                        bias=mgc[:], scale=-1.0,
                    )
                    r_t = work.tile([P, J], f32, tag="r")
                    nc.vector.scalar_tensor_tensor(
                        r_t[:], w2_t[:], MAGIC, u_t[:], A.subtract, A.add
                    )
                    prev_next = (r_t, c)
                else:
                    prev_next = None

                if prev is not None:
                    r_p, cp = prev
                    s_t = outp.tile([P, J], f32, tag="s")
                    nc.scalar.activation(
                        s_t[:], r_p[:], mybir.ActivationFunctionType.Sin,
                        bias=zro[:], scale=TWO_PI,
                    )
                    nc.sync.dma_start(
                        out_t[0:VLEAD, cp * J : (cp + 1) * J], s_t[:]
                    )
                    # spread the 4 big replicate DMAs across the run on the
                    # scalar HWDGE ring so both rings share HBM evenly
                    if cp % rep_every == 0 and cp // rep_every < NREP:
                        rr = cp // rep_every
                        r0 = VLEAD + rr * rep_rows
                        nc.scalar.dma_start(
                            out_v[:, r0 : r0 + rep_rows, :],
                            s127[:]
                            .unsqueeze(1)
                            .to_broadcast([P, rep_rows, NX // P]),
                        )
                prev = prev_next

    nc.compile()
    return nc


def _split3(a: np.ndarray):
    """Exact-ish 3-way bf16 split: h + m + l == a to within ~2^-25 rel."""
    import ml_dtypes

    bf = ml_dtypes.bfloat16
    h = a.astype(bf)
    m = (a - h.astype(np.float32)).astype(bf)
    l = (a - h.astype(np.float32) - m.astype(np.float32)).astype(bf)
    return h, m, l


def _in_maps(x: np.ndarray):
    """Per-core input dict. The 6 bf16 K-rows pair as
    (fh,xh) (fh,xm) (fh,xl) (fm,xh) (fm,xm) (fl,xh) — every partial
    product of magnitude >= |u| * 2^-24."""
    import ml_dtypes

    fp = _freqs_over_2pi()
    fh, fm, fl = _split3(fp)
    f_rows = np.stack([fh, fh, fh, fm, fm, fl])  # [KSP, 128] bf16
    f6 = np.ascontiguousarray(
        np.tile(f_rows, (XROWS, 1)).astype(ml_dtypes.bfloat16)
    )
    in_maps = []
    for c in range(NCORES):
        xs = x[c * B_PER_CORE : (c + 1) * B_PER_CORE].reshape(-1)
        xh, xm, xl = _split3(xs.reshape(XROWS, XCOL))
        x6 = np.ascontiguousarray(
            np.stack([xh[0], xm[0], xl[0], xh[0], xm[0], xh[0],
                      xh[1], xm[1], xl[1], xh[1], xm[1], xh[1]])
        )
        in_maps.append(
            {
                "x6": x6,
                "x128": np.ascontiguousarray(xs.reshape(P, NX // P)),
                "f6": f6,
            }
        )
    return in_maps


def kernel(x, vector_size):
    global _BUILT
    x = np.asarray(x, dtype=np.float32)
    assert x.shape == (B, N), x.shape
    assert int(vector_size) == V, vector_size

    if _BUILT is None:
        _BUILT = _build(_freqs_over_2pi())
    nc = _BUILT

    res = run_bass_kernel_spmd(nc, _in_maps(x), list(range(NCORES)))

    out = np.empty((B, N, V), dtype=np.float32)
    for c in range(NCORES):
        oc = res.results[c]["out"]  # [256, 32768] freq-major
        out[c * B_PER_CORE : (c + 1) * B_PER_CORE] = np.ascontiguousarray(
            oc.T
        ).reshape(B_PER_CORE, N, V)
    return out


# revision 10
# speedup vs baseline: 4.4128x; 1.0072x over previous
"""Trainium2 Bass kernel for nn_InputLayer_57337813401914.

out[b, n, v] = sin(x[b, n] * freqs[v]), x: [64, 4096] f32,
freqs[v] = 10 ** (min(v, 127) / 127 * 4), v in [0, 256).

Sharding: batch dim (64) split across 8 NeuronCores; per core 32768 x
values, device output freq-major [256, 32768] (host transposes back).

Per-core pipeline (partition = frequency):
  PE    u[v, j]  = (freqs[v]/2pi) * x[j]      64 outer products -> PSUM
  ACT   w2       = Identity(-u + MAGIC)       = MAGIC - rint(u) (magic trick)
  DVE   r        = (w2 - MAGIC) + u           = u - rint(u) in [-0.5, 0.5]
  ACT   s        = Sin(2pi * r)               table exact on [-pi, pi]
  DMA   out[v, chunk] <- s                    sync ring
Rows 127..255 all equal sin(1e4 * x): a tiny [128, 256]-layout side
pipeline computes s127 once and four broadcast DMAs on the scalar ring
replicate it into rows 128..255, overlapped with the main chunk loop.
Single f32 multiply range reduction: rel err vs reference ~2.7e-4
(tolerance 2e-2); the magic-number rint and the exact Sterbenz
subtractions make r = frac-reduction of fl(x*f/2pi) exact.
"""
import numpy as np
from contextlib import ExitStack

import concourse.bacc as bacc
import concourse.tile as tile
from concourse import mybir
from concourse.alu_op_type import AluOpType as A
from concourse.bass_utils import run_bass_kernel_spmd

P = 128            # SBUF/PSUM partitions; also the number of distinct freqs
NX = 32768         # x values per core
VLEAD = 128        # computed frequency rows (rows 128..255 replicate row 127)
V = 256            # total output rows
J = 2048           # x chunk per pipeline stage (free dim)
NCHUNK = NX // J   # 16
MMN = 512          # moving free dim per fp32 matmul (PE limit / PSUM bank)
NMM = J // MMN     # 4 matmuls per chunk
XROWS = 2          # x halves live on partitions 0 and 32 (matmul base rule)
XCOL = NX // XROWS
KSP = 6            # bf16 split products per outer product (see _split6)
NREP = 8           # replicate DMA count for rows 128..255
NCORES = 8
B, N = 64, 4096
B_PER_CORE = B // NCORES

MAGIC = float(np.float32(1.5 * 2**23))
TWO_PI = float(np.float32(2.0 * np.pi))

_BUILT = None


def _freqs_lead() -> np.ndarray:
    """First 128 freqs, bit-matching the reference (jnp f32 ops)."""
    try:
        import jax.numpy as jnp

        f = 10.0 ** (jnp.arange(VLEAD, dtype=jnp.float32) / (VLEAD - 1) * 4.0)
        return np.asarray(f, dtype=np.float32)
    except Exception:
        y = np.arange(VLEAD, dtype=np.float32) / np.float32(VLEAD - 1)
        y = y * np.float32(4.0)
        return np.power(np.float32(10.0), y, dtype=np.float32)


def _freqs_over_2pi() -> np.ndarray:
    return (
        _freqs_lead().astype(np.float64) / (2.0 * np.pi)
    ).astype(np.float32)


def _build(fp: np.ndarray):
    nc = bacc.Bacc(
        "TRN2", target_bir_lowering=False, debug=False, num_devices=NCORES
    )
    f32 = mybir.dt.float32
    bf16 = mybir.dt.bfloat16
    x6_in = nc.dram_tensor(
        "x6", [XROWS * KSP, XCOL], bf16, kind="ExternalInput"
    ).ap()
    x128_in = nc.dram_tensor("x128", [P, NX // P], f32, kind="ExternalInput").ap()
    f6_in = nc.dram_tensor(
        "f6", [XROWS * KSP, P], bf16, kind="ExternalInput"
    ).ap()
    out_t = nc.dram_tensor("out", [V, NX], f32, kind="ExternalOutput").ap()
    # out[v, p*256 + i] viewed partition-first: replicate-DMA destination
    out_v = out_t.rearrange("v (p i) -> p v i", p=P)

    fp127 = float(fp[VLEAD - 1])

    with tile.TileContext(nc) as tc:
        with ExitStack() as ctx:
            const = ctx.enter_context(tc.tile_pool(name="const", bufs=1))
            psum = ctx.enter_context(
                tc.tile_pool(name="psum", bufs=2, space="PSUM")
            )
            work = ctx.enter_context(tc.tile_pool(name="work", bufs=3))
            outp = ctx.enter_context(tc.tile_pool(name="outp", bufs=4))

            x6 = const.tile([32 + KSP, XCOL], bf16, tag="x6")
            x128 = const.tile([P, NX // P], f32, tag="x128")
            f6 = const.tile([32 + KSP, P], bf16, tag="f6")
            nc.sync.dma_start(x6[0:KSP, :], x6_in[0:KSP, :])
            nc.sync.dma_start(x6[32 : 32 + KSP, :], x6_in[KSP : 2 * KSP, :])
            nc.sync.dma_start(x128[:], x128_in[:])
            nc.sync.dma_start(f6[0:KSP, :], f6_in[0:KSP, :])
            nc.sync.dma_start(f6[32 : 32 + KSP, :], f6_in[KSP : 2 * KSP, :])
            # per-partition MAGIC column for the activation bias operand
            mgc = const.tile([P, 1], f32, tag="magic")
            nc.vector.memset(mgc[:], MAGIC)
            zro = const.tile([P, 1], f32, tag="zero")
            nc.vector.memset(zro[:], 0.0)

            # side pipeline: s127[p, i] = sin(1e4 * x[p*256+i]) feeds the
            # broadcast replicates for rows 128..255
            u7 = work.tile([P, NX // P], f32, tag="u7")
            nc.vector.tensor_scalar(u7[:], x128[:], fp127, None, A.mult)
            w7 = work.tile([P, NX // P], f32, tag="w7")
            nc.vector.tensor_scalar(
                w7[:], u7[:], -1.0, MAGIC, A.mult, A.add
            )
            r7 = work.tile([P, NX // P], f32, tag="r7")
            nc.vector.scalar_tensor_tensor(
                r7[:], w7[:], MAGIC, u7[:], A.subtract, A.add
            )
            s127 = const.tile([P, NX // P], f32, tag="s127")
            nc.scalar.activation(
                s127[:], r7[:], mybir.ActivationFunctionType.Sin,
                bias=zro[:], scale=TWO_PI,
            )


            rep_rows = (V - VLEAD) // NREP
            rep_every = NCHUNK // NREP

            # software-pipelined main loop: ACT queue order is
            # w2(0), w2(1), sin(0), w2(2), sin(1), ... so the per-chunk
            # ACT->DVE->ACT chain overlaps across chunks.
            prev = None  # (r_tile, u_tile, c) pending sin+store
            for c in range(NCHUNK + 1):
                if c < NCHUNK:
                    u_t = psum.tile([P, J], f32, tag="u")
                    for m in range(NMM):
                        g = c * NMM + m
                        row = 32 * (g // (XCOL // MMN))
                        col = (g % (XCOL // MMN)) * MMN
                        nc.tensor.matmul(
                            u_t[:, m * MMN : (m + 1) * MMN],
                            f6[row : row + KSP, :],
                            x6[row : row + KSP, col : col + MMN],
                            start=True,
                            stop=True,
                        )
                    w2_t = work.tile([P, J], f32, tag="w2")
                    nc.vector.tensor_scalar(
                        w2_t[:], u_t[:], -1.0, MAGIC, A.mult, A.add
                    )
                    r_t = work.tile([P, J], f32, tag="r")
                    nc.vector.scalar_tensor_tensor(
                        r_t[:], w2_t[:], MAGIC, u_t[:], A.subtract, A.add
                    )
                    prev_next = (r_t, c)
                else:
                    prev_next = None

                if prev is not None:
                    r_p, cp = prev
                    s_t = outp.tile([P, J], f32, tag="s")
                    nc.scalar.activation(
                        s_t[:], r_p[:], mybir.ActivationFunctionType.Sin,
                        bias=zro[:], scale=TWO_PI,
                    )
                    nc.sync.dma_start(
                        out_t[0:VLEAD, cp * J : (cp + 1) * J], s_t[:]
                    )
                    # spread the replicate DMAs across the run on the
                    # scalar HWDGE ring (ACT only runs Sin now, ~25% busy)
                    if cp % rep_every == 0 and cp // rep_every < NREP:
                        rr = cp // rep_every
                        r0 = VLEAD + rr * rep_rows
                        nc.scalar.dma_start(
                            out_v[:, r0 : r0 + rep_rows, :],
                            s127[:]
                            .unsqueeze(1)
                            .to_broadcast([P, rep_rows, NX // P]),
                        )
                prev = prev_next

    nc.compile()
    return nc


def _split3(a: np.ndarray):
    """Exact-ish 3-way bf16 split: h + m + l == a to within ~2^-25 rel."""
    import ml_dtypes

    bf = ml_dtypes.bfloat16
    h = a.astype(bf)
    m = (a - h.astype(np.float32)).astype(bf)
    l = (a - h.astype(np.float32) - m.astype(np.float32)).astype(bf)
    return h, m, l


def _in_maps(x: np.ndarray):
    """Per-core input dict. The 6 bf16 K-rows pair as
    (fh,xh) (fh,xm) (fh,xl) (fm,xh) (fm,xm) (fl,xh) — every partial
    product of magnitude >= |u| * 2^-24."""
    import ml_dtypes

    fp = _freqs_over_2pi()
    fh, fm, fl = _split3(fp)
    f_rows = np.stack([fh, fh, fh, fm, fm, fl])  # [KSP, 128] bf16
    f6 = np.ascontiguousarray(
        np.tile(f_rows, (XROWS, 1)).astype(ml_dtypes.bfloat16)
    )
    in_maps = []
    for c in range(NCORES):
        xs = x[c * B_PER_CORE : (c + 1) * B_PER_CORE].reshape(-1)
        xh, xm, xl = _split3(xs.reshape(XROWS, XCOL))
        x6 = np.ascontiguousarray(
            np.stack([xh[0], xm[0], xl[0], xh[0], xm[0], xh[0],
                      xh[1], xm[1], xl[1], xh[1], xm[1], xh[1]])
        )
        in_maps.append(
            {
                "x6": x6,
                "x128": np.ascontiguousarray(xs.reshape(P, NX // P)),
                "f6": f6,
            }
        )
    return in_maps


def kernel(x, vector_size):
    global _BUILT
    x = np.asarray(x, dtype=np.float32)
    assert x.shape == (B, N), x.shape
    assert int(vector_size) == V, vector_size

    if _BUILT is None:
        _BUILT = _build(_freqs_over_2pi())
    nc = _BUILT

    res = run_bass_kernel_spmd(nc, _in_maps(x), list(range(NCORES)))

    out = np.empty((B, N, V), dtype=np.float32)
    for c in range(NCORES):
        oc = res.results[c]["out"]  # [256, 32768] freq-major
        out[c * B_PER_CORE : (c + 1) * B_PER_CORE] = np.ascontiguousarray(
            oc.T
        ).reshape(B_PER_CORE, N, V)
    return out
